# revision 22
# baseline (speedup 1.0000x reference)
"""Causal single-head attention (B=4, T=4096, D=1024) on 8 trn2 NeuronCores.

Sharding: 2 cores per batch element, split by key-block PARITY (flash-style):
  core = 2*b + p ; p in {0,1}
  Each core computes, for ALL 4096 queries of batch b, the partial
  (unnormalized) attention output over its 16 key blocks {128*(2u+p)} and the
  partial softmax row-sums. Host merges: O = (O_0 + O_1) / (rs_0 + rs_1).

v4 design (on top of v3's M-fold + bf16 + parity split):
  Key-side M-fold: scores = q.k^T = x Wq^T Wk x̃^T = x (x̃ M^T)^T with
    M = Wq^T Wk precomputed on host; no q projection at all.
  Single interleaved PE stream: attention chunks ascend j = 0..15 with the
    kproj / vproj groups slotted between the small early chunks as filler,
    so every chunk's PSUM drain hides behind real PE work (v3 ran the
    projections up front and stalled ~3 us per small chunk at the end).
  Diagonal score block first within each chunk: the mask-add + exp serial
    chain lands at the start of the chunk where it overlaps scoring, not
    after the last matmul.
  Last chunk split into 2x128-query halves on disjoint PSUM banks: the
    final drain is 0.5 MB instead of 1 MB and half of it hides behind the
    second half's compute.
  O emitted bf16 (host merge upcasts): halves output DMA; each drain is a
    single flat 256 KB descriptor (row-contiguous [128, 1024] bf16).
  Startup: ig-major MT layout + per-cb kt slices so the first kproj matmul
    needs only 384 KB of DMA, not 3 MB.
"""

import sys

sys.path.insert(0, "/opt/trn_rl_repo")

import numpy as np
import ml_dtypes
from contextlib import ExitStack

import concourse.tile as tile
from concourse import bacc, mybir
from concourse.bass_utils import run_bass_kernel_spmd

P = 128
D = 1024
T = 4096
B = 4
NDB = D // P  # 8 feature blocks
NCB = D // P  # 8 contraction blocks
NKB = 16  # key blocks per core (parity half of 32)
QC = 256  # query-chunk columns
NQC = T // QC  # 16
F32 = mybir.dt.float32
BF16 = mybir.dt.bfloat16
EXPSCALE = 1.0 / 32.0  # 1/sqrt(D)
EXP = mybir.ActivationFunctionType.Exp

_CACHED_NC = None
_LAST_RES = None


def _build_program():
    nc = bacc.Bacc("TRN2", target_bir_lowering=False, debug=False, num_devices=8)

    # All bulk inputs pre-packed host-side so every DMA is a single 2-D
    # descriptor plane (descriptor issue costs ~0.6us per plane, serialized
    # per queue).
    xq_d = nc.dram_tensor("XQ", [NQC, P, NCB, QC], BF16, kind="ExternalInput").ap()
    kt_d = nc.dram_tensor("KT", [4, P, NCB, 512], BF16, kind="ExternalInput").ap()
    m_d = nc.dram_tensor("MT2", [NDB, P, NCB, P], BF16, kind="ExternalInput").ap()
    wv_d = nc.dram_tensor("WV", [2, P, NCB, 512], BF16, kind="ExternalInput").ap()
    mask_d = nc.dram_tensor("mask", [P, QC], F32, kind="ExternalInput").ap()
    o_d = nc.dram_tensor("O", [T, D], BF16, kind="ExternalOutput").ap()
    rs_d = nc.dram_tensor("rs", [T, 1], F32, kind="ExternalOutput").ap()

    xq_r = xq_d.rearrange("j p a c -> p j a c")  # [128, 16, 8, 256]
    kt_r = kt_d.rearrange("g p a c -> p g a c")  # [128, 4, 8, 512]
    m_r = m_d.rearrange("g p a c -> p g a c")  # [128, ig 8, cb 8, 128]
    wv_r = wv_d.rearrange("v p a c -> p v a c")  # [128, 2, 8, 512]

    with tile.TileContext(nc) as tc, ExitStack() as ctx:
        kv = ctx.enter_context(tc.tile_pool(name="kv", bufs=1))
        xp = ctx.enter_context(tc.tile_pool(name="xp", bufs=4))
        wp = ctx.enter_context(tc.tile_pool(name="wp", bufs=2))
        pp = ctx.enter_context(tc.tile_pool(name="pp", bufs=4))
        stg = ctx.enter_context(tc.tile_pool(name="stg", bufs=4))
        psum = ctx.enter_context(tc.tile_pool(name="psum", bufs=1, space="PSUM"))

        mask_t = kv.tile([P, QC], F32, tag="mask")
        mT_t = kv.tile([P, NDB, NCB, P], BF16, tag="mT")  # M^T ig-major, 16 KiB
        kt_t = kv.tile([P, 4, NCB, 512], BF16, tag="kt")  # x̃^T g-slabs, 32 KiB
        kpT_t = kv.tile([P, NCB, T // 2], BF16, tag="kpT")  # k'^T, 32 KiB
        v_t = kv.tile([P, NKB, D + 4], BF16, tag="vt")  # 32.1 KiB

        # ---- startup DMAs, issues spread over the three DMA-capable queues
        # (sync / scalar / gpsimd; ~0.6us serialized issue per descriptor) ----
        # sync: the transfers gating the first kproj matmuls, then the xq feed.
        nc.sync.dma_start(mT_t[:, 0], m_r[:, 0])
        nc.sync.dma_start(kt_t[:, 0, 0:4], kt_r[:, 0, 0:4])
        nc.sync.dma_start(kt_t[:, 0, 4:8], kt_r[:, 0, 4:8])
        # rowsum ones-columns via strided memsets (no DMA descriptors)
        nc.vector.memset(v_t[:, :, D : D + 1], 1.0)
        nc.vector.memset(v_t[:, :, D + 1 : D + 4], 0.0)
        # scalar: rest of M^T, then wv (needed at vproj(0), ~+14us).
        for ig in range(1, NDB):
            nc.scalar.dma_start(mT_t[:, ig], m_r[:, ig])
        wvs = []
        for vc in range(2):
            wv = wp.tile([P, NCB, 512], BF16, tag="wv", name=f"wv{vc}")
            nc.scalar.dma_start(wv[:], wv_r[:, vc])
            wvs.append(wv)
        # gpsimd: remaining key slabs + mask (drains only start ~+20us).
        for g in (1, 2, 3):
            nc.gpsimd.dma_start(kt_t[:, g], kt_r[:, g])
        nc.gpsimd.dma_start(mask_t[:], mask_d[:])

        xqs = {}

        def fetch(j):
            if j <= 15 and j not in xqs:
                t = xp.tile([P, NCB, QC], BF16, tag="x", name=f"xq{j}")
                nc.sync.dma_start(t[:], xq_r[:, j])
                xqs[j] = t

        fetch(0)
        fetch(1)
        fetch(2)

        prot = [0]  # kproj/vproj PSUM double-buffer rotation

        def kproj(g):
            for ig in range(NDB):
                ps = psum.tile([P, 512], F32, tag=f"p{prot[0] % 2}", name=f"kps{g}_{ig}")
                prot[0] += 1
                for cb in range(NCB):
                    nc.tensor.matmul(
                        ps[:],
                        mT_t[:, ig, cb, :],
                        kt_t[:, g, cb, :],
                        start=(cb == 0),
                        stop=(cb == NCB - 1),
                    )
                nc.vector.tensor_copy(kpT_t[:, ig, g * 512 : (g + 1) * 512], ps[:])

        def vproj(kb):
            for vc in range(2):
                ps = psum.tile([P, 512], F32, tag=f"p{prot[0] % 2}", name=f"vps{vc}_{kb}")
                prot[0] += 1
                for cb in range(NCB):
                    nc.tensor.matmul(
                        ps[:],
                        kt_t[:, kb // 4, cb, (kb % 4) * P : (kb % 4 + 1) * P],
                        wvs[vc][:, cb, :],
                        start=(cb == 0),
                        stop=(cb == NCB - 1),
                    )
                nc.vector.tensor_copy(v_t[:, kb, vc * 512 : (vc + 1) * 512], ps[:])

        def attn(j, qoff, qn, btag0, rr=None):
            """Score+AV for chunk j over xq columns [qoff, qoff+qn).

            btag0: first of 2*(qn//128) consecutive acc bank tags (b0..b3).
            rr: optional existing row-sum bank tile (shared by the two
            half-chunks of j=15). Returns (acc, qoff, nsub, rr)."""
            fetch(j + 2)
            xq = xqs[j]
            if j != 15 or qoff == P:
                xqs.pop(j)
            nsub = qn // P
            roff = btag0 // 2
            if rr is None:
                rr = psum.tile([P, 8], F32, tag="rr", name=f"rr_{j}_{qoff}")
            ss = psum.tile([P, 512], F32, tag="ss", name=f"ss_{j}_{qoff}")
            acc = {}
            for sub in range(nsub):
                acc[sub, 0] = psum.tile(
                    [P, 512], F32, tag=f"b{btag0 + 2 * sub}", name=f"a0_{j}_{qoff}_{sub}"
                )
                acc[sub, 1] = psum.tile(
                    [P, 512], F32, tag=f"b{btag0 + 2 * sub + 1}", name=f"a1_{j}_{qoff}_{sub}"
                )
                acc[sub, 2] = rr[:, 4 * (roff + sub) : 4 * (roff + sub) + 4]

            def av(u, pt_t, first, last):
                for sub in range(nsub):
                    lhs = pt_t[:, sub * P : (sub + 1) * P]
                    nc.tensor.matmul(
                        acc[sub, 0][:], lhs, v_t[:, u, 0:512],
                        start=first, stop=last, skip_group_check=True,
                    )
                    nc.tensor.matmul(
                        acc[sub, 1][:], lhs, v_t[:, u, 512:1024],
                        start=first, stop=last, skip_group_check=True,
                    )
                    # rowsum groups share one PSUM bank; start marks the WHOLE
                    # 2KB bank pending-zero, so only sub0 may issue it — sub1's
                    # first write lazily zeroes its own region off that mark.
                    nc.tensor.matmul(
                        acc[sub, 2], lhs, v_t[:, u, D : D + 4],
                        start=first and sub == 0, stop=last, skip_group_check=True,
                    )

            # Diagonal block SECOND: its mask-add -> exp chain (gated on the
            # previous chunk's drain CASTs on Vector) overlaps block 0's
            # scoring instead of stalling the score stream at chunk start.
            uorder = [j] if j == 0 else [0, j] + list(range(1, j))
            pts = {}
            for i, u in enumerate(uorder):
                st = ss[:, (i % 2) * 256 : (i % 2) * 256 + qn]
                for db in range(NDB):
                    nc.tensor.matmul(
                        st,
                        kpT_t[:, db, u * P : (u + 1) * P],
                        xq[:, db, qoff : qoff + qn],
                        start=(db == 0),
                        stop=(db == NDB - 1),
                    )
                if u == j:
                    nc.vector.tensor_add(st, st, mask_t[:, qoff : qoff + qn])
                pt = pp.tile([P, qn], BF16, tag="pt", name=f"pt{j}_{qoff}_{u}")
                nc.scalar.activation(pt[:], st, EXP, scale=EXPSCALE)
                pts[u] = pt
                if i >= 2:
                    av(uorder[i - 2], pts.pop(uorder[i - 2]),
                       first=(i == 2), last=False)
            n = len(uorder)
            if n >= 2:
                av(uorder[n - 2], pts.pop(uorder[n - 2]), first=(n == 2), last=False)
            av(uorder[n - 1], pts.pop(uorder[n - 1]), first=(n == 1), last=True)
            return acc, qoff, nsub, rr

        def drain(acc, qoff, nsub, j, final=False):
            # Mid-stream drains put BOTH output copies on Vector so the Scalar
            # queue stays clear for the next chunk's exps (a front-loaded
            # Scalar copy delays exp(i0) and stalls the score pipeline). The
            # final drain splits them Vector/Scalar for latency.
            dma = nc.sync.dma_start if final else nc.gpsimd.dma_start
            for sub in range(nsub):
                row = j * QC + qoff + sub * P
                big = stg.tile([P, D], BF16, tag="stage", name=f"ot_{j}_{qoff}_{sub}")
                nc.vector.tensor_copy(big[:, 0:512], acc[sub, 0][:])
                if final:
                    nc.scalar.copy(big[:, 512:1024], acc[sub, 1][:])
                else:
                    nc.vector.tensor_copy(big[:, 512:1024], acc[sub, 1][:])
                rt = stg.tile([P, 1], F32, tag="rt", name=f"rt{j}_{qoff}_{sub}")
                nc.scalar.copy(rt[:], acc[sub, 2][:, 0:1])
                dma(o_d[row : row + P, :], big[:])
                dma(rs_d[row : row + P, :], rt[:])

        # ---- interleaved PE stream ----
        kproj(0)
        vproj(0)
        vproj(1)
        FILLER = {
            0: [("vp", 2), ("vp", 3)],
            1: [("vp", 4), ("vp", 5)],
            2: [("vp", 6), ("vp", 7)],
            3: [("kp", 1)],
            4: [("vp", 8), ("vp", 9)],
            5: [("vp", 10), ("vp", 11)],
            6: [("kp", 2)],
            7: [("vp", 12), ("vp", 13)],
            8: [("vp", 14), ("vp", 15)],
            9: [("kp", 3)],
        }
        for j in range(15):
            acc, qoff, nsub, rr = attn(j, 0, QC, 0)
            drain(acc, qoff, nsub, j)
            for op, arg in FILLER.get(j, []):
                kproj(arg) if op == "kp" else vproj(arg)
        acc, qoff, nsub, rr = attn(15, 0, P, 0)
        drain(acc, qoff, nsub, 15)
        acc, qoff, nsub, _ = attn(15, P, P, 2, rr=rr)
        drain(acc, qoff, nsub, 15, final=True)

    nc.finalize()
    return nc


def _get_program():
    global _CACHED_NC
    if _CACHED_NC is None:
        _CACHED_NC = _build_program()
    return _CACHED_NC


def _masks():
    neg = np.float32(-1e30)
    tri = np.where(np.triu(np.ones((P, P), dtype=bool)), np.float32(0), neg)
    keep = np.zeros((P, P), dtype=np.float32)
    drop = np.full((P, P), neg, dtype=np.float32)
    return (
        np.ascontiguousarray(np.concatenate([tri, keep], axis=1)),  # even core
        np.ascontiguousarray(np.concatenate([drop, tri], axis=1)),  # odd core
    )


def kernel(x, Wq, Wk, Wv):
    out, _ = _run(x, Wq, Wk, Wv, trace=False)
    return out


def _run(x, Wq, Wk, Wv, trace=False, keep_res=False):
    BF = ml_dtypes.bfloat16
    x = np.asarray(x, dtype=np.float32)
    M = (np.asarray(Wq, np.float64).T @ np.asarray(Wk, np.float64)).astype(np.float32)
    A = np.ascontiguousarray(M.T.astype(BF))  # [j, i]
    # [ig, p(=j in cb), cb, c(=i in ig)]
    MT2 = np.ascontiguousarray(
        A.reshape(NCB, P, NDB, P).transpose(2, 1, 0, 3)
    )
    WvT_bf = np.asarray(Wv, np.float32).T.astype(BF)
    m_even, m_odd = _masks()

    WV2 = np.ascontiguousarray(WvT_bf.reshape(NCB, P, 2, 512).transpose(2, 1, 0, 3))
    nc = _get_program()
    in_maps = []
    for core in range(8):
        b, p = core // 2, core % 2
        xT = x[b].T.astype(BF)  # [D, T]
        xTk = xT.reshape(D, T // P, P)[:, p::2, :].reshape(D, T // 2)
        # [j, p, a(feature blk), c] / [g, p, cb, c] plane-packed
        XQ = np.ascontiguousarray(xT.reshape(NCB, P, NQC, QC).transpose(2, 1, 0, 3))
        KT = np.ascontiguousarray(xTk.reshape(NCB, P, 4, 512).transpose(2, 1, 0, 3))
        in_maps.append(
            {
                "XQ": XQ,
                "KT": KT,
                "MT2": MT2,
                "WV": WV2,
                "mask": m_even if p == 0 else m_odd,
            }
        )

    res = run_bass_kernel_spmd(nc, in_maps, core_ids=list(range(8)), trace=trace)
    if keep_res:
        global _LAST_RES
        _LAST_RES = res
    out = np.empty((B, T, D), dtype=np.float32)
    for b in range(B):
        O0 = np.asarray(res.results[2 * b]["O"], dtype=np.float32)
        O1 = np.asarray(res.results[2 * b + 1]["O"], dtype=np.float32)
        rs0, rs1 = res.results[2 * b]["rs"], res.results[2 * b + 1]["rs"]
        out[b] = (O0 + O1) / (rs0 + rs1)
    return out, res.exec_time_ns
